# revision 33
# baseline (speedup 1.0000x reference)
"""CKAM (DANet-style dual attention) Bass kernel for 8 trn2 NeuronCores.

Data-parallel over batch: each core processes one [512, 64, 64] image.

Per-core dataflow (N = H*W = 4096, C = 512, CH = 256, R = 64):
  Head (column-streamed, overlaps the input DMA):
    per 512-col m-block b:
      A:  qk[:, b]   = Wsp^T @ x[:, b]          (spatial q|k, channel-major)
      B:  [qc|kc|ks]^T[b-rows] = x[:, b]^T @ Wcsc   (n-major)
      chan scores += qcT_nb^T @ kcT_nb          (PSUM-resident accumulation)
      kc rows of `stacked` via DMA-transpose of kcT
      qk_swap[:, b] partition-swapped dup of qk (SBUF-SBUF DMA)
  Chan tail: softmax(scores) -> ac; final weights folded on device:
      wf2 rows for the kc half become (fc @ ac)^T = ac^T @ fc^T  (1 matmul)
  Spatial attn: chunk pairs (even chunk on PE rows 0:64, odd on 64:128):
      S = q^T k -> exp (ACT, accum d; optionally DVE Horner poly for some
      tiles) -> out_sp += (ks^T / d) contracted with E (col-group pairs)
  Final: out = wf2 @ [out_sp | kc] + bias   (single K=128 bf16 conv), bf16 out.

All 1x1 convs are folded through the (never materialized) x = top+bottom:
composite weights are computed on the host in float64.
"""

import numpy as np

import concourse.bass as bass
import concourse.bacc as bacc
import concourse.mybir as mybir
import concourse.tile as tile
from concourse import bass_utils
from concourse.bass import ts
from concourse.alu_op_type import AluOpType
from concourse.masks import make_identity

N_CORES = 8
C, HW = 512, 4096
CH, R = 256, 64
F32 = mybir.dt.float32
BF16 = mybir.dt.bfloat16
EXP = mybir.ActivationFunctionType.Exp
AX = mybir.AxisListType.X

_CACHE: dict = {}

# which (o-parity) q-slots use the DVE polynomial exp instead of ScalarE ACT
# (measured: DVE STT runs at 1x here => poly is 5x slower than ACT; keep off)
DVE_EXP_QS: tuple = ()

# exp(x) ~ poly deg-5, density(sigma=0.4)-weighted fit with 1e-3 floor,
# valid on [-2.9, 2.9] (S scores are ~N(0, 0.35^2), |S| < 2.5 in practice)
def _exp_poly_coeffs():
    a = 2.9
    x = np.linspace(-a, a, 12001)
    dens = np.exp(-(x ** 2) / (2 * 0.40 ** 2))
    w = np.maximum(dens, 1e-3) / np.exp(x)
    c = np.polyfit(x, np.exp(x), 5, w=np.sqrt(w))
    return tuple(float(v) for v in c)  # highest degree first


def build_program(repeat=1, dve_qs=None):
    if dve_qs is None:
        dve_qs = DVE_EXP_QS
    c5, c4, c3, c2, c1, c0 = _exp_poly_coeffs()
    nc = bacc.Bacc("TRN2", target_bir_lowering=False, debug=False)

    top = nc.dram_tensor("top", (C, HW), BF16, kind="ExternalInput").ap()
    bot = nc.dram_tensor("bot", (C, HW), BF16, kind="ExternalInput").ap()
    wsp = nc.dram_tensor("wsp", (128, 8, 128), BF16, kind="ExternalInput").ap()
    wcsc = nc.dram_tensor("wcsc", (128, 8, 192), BF16, kind="ExternalInput").ap()
    wfa = nc.dram_tensor("wfa", (128, 4, 128), BF16, kind="ExternalInput").ap()
    wfb = nc.dram_tensor("wfb", (128, 4, 128), BF16, kind="ExternalInput").ap()
    b_qk = nc.dram_tensor("b_qk", (128, 1), F32, kind="ExternalInput").ap()
    b_csc = nc.dram_tensor("b_csc", (128, 192), F32, kind="ExternalInput").ap()
    b_f = nc.dram_tensor("b_f", (128, 4), F32, kind="ExternalInput").ap()
    out_d = nc.dram_tensor("out", (C, HW), BF16, kind="ExternalOutput").ap()

    with tile.TileContext(nc) as tc:
      for _rep in range(repeat):
        with (
            tc.tile_pool(name="consts", bufs=1) as consts,
            tc.tile_pool(name="persist", bufs=1) as persist,
        ):
            wsp_sb = consts.tile([128, 8, 128], BF16)
            nc.sync.dma_start(out=wsp_sb, in_=wsp)
            wcsc_sb = consts.tile([128, 8, 192], BF16)
            nc.sync.dma_start(out=wcsc_sb, in_=wcsc)
            bqk_sb = consts.tile([128, 1], F32)
            nc.sync.dma_start(out=bqk_sb, in_=b_qk)
            bcsc_sb = consts.tile([128, 192], F32)
            nc.sync.dma_start(out=bcsc_sb, in_=b_csc)
            wfa_sb = consts.tile([128, 4, 128], BF16)
            wfb_sb = consts.tile([128, 4, 128], BF16)
            bf_sb = consts.tile([128, 4], F32)
            fcTa = consts.tile([64, 4, 128], BF16)
            fcTb = consts.tile([64, 4, 128], BF16)
            ident = consts.tile([128, 128], BF16)
            make_identity(nc, ident)

            qk_sb = persist.tile([128, HW], BF16)  # q rows 0:64, k rows 64:128
            qk_swap = persist.tile([128, HW], BF16)  # [k | q] partition-swapped
            qckcT = persist.tile([128, 32, 128], BF16)  # qc^T | kc^T (n-major)
            ksT = persist.tile([128, 32, 64], BF16)  # spatial k^T
            stacked = persist.tile([128, HW], BF16)  # [out_sp | kc] final input
            wf2a = persist.tile([128, 4, 128], BF16)  # attn-folded final w
            wf2b = persist.tile([128, 4, 128], BF16)

            # ---------------- column-streamed head ----------------
            with (
                tc.tile_pool(name="inputs", bufs=1) as inputs,
                tc.tile_pool(name="psA", bufs=2, space="PSUM") as psA,
                tc.tile_pool(name="psB", bufs=2, space="PSUM") as psB,
                tc.tile_pool(name="psSc", bufs=1, space="PSUM") as psSc,
                tc.tile_pool(name="psW", bufs=1, space="PSUM") as psW,
                tc.tile_pool(name="psT", bufs=2, space="PSUM") as psT,
                tc.tile_pool(name="chs", bufs=1) as chs,
            ):
                top_c = top.rearrange("(a p) m -> p a m", p=128)
                bot_c = bot.rearrange("(a p) m -> p a m", p=128)
                top_sb = inputs.tile([128, 4, HW], BF16, name="top_sb")
                bot_sb = inputs.tile([128, 4, HW], BF16, name="bot_sb")
                chunks = [top_sb[:, a, :] for a in range(4)] + [
                    bot_sb[:, a, :] for a in range(4)
                ]
                sc_ps = psSc.tile([64, 64], F32, tag="sc")

                # input stream: one big DMA per (tensor, block), two HWDGE
                # queues in parallel, no compute interleaved in the queues
                for b in range(8):
                    nc.sync.dma_start(
                        out=top_sb[:, :, ts(b, 512)], in_=top_c[:, :, ts(b, 512)]
                    )
                    nc.scalar.dma_start(
                        out=bot_sb[:, :, ts(b, 512)], in_=bot_c[:, :, ts(b, 512)]
                    )
                # late-needed weights, behind the input streams
                nc.sync.dma_start(out=wfa_sb, in_=wfa)
                nc.sync.dma_start(out=wfb_sb, in_=wfb)
                nc.sync.dma_start(out=bf_sb, in_=b_f)
                nc.sync.dma_start(out=fcTa, in_=wfa_sb[64:128, :, :])
                nc.sync.dma_start(out=fcTb, in_=wfb_sb[0:64, :, :])

                for b in range(8):
                    # A: q|k conv for this column block
                    pa = psA.tile([128, 512], F32, tag="a", name="pa")
                    for ci in range(8):
                        nc.tensor.matmul(
                            pa,
                            wsp_sb[:, ci, :],
                            chunks[ci][:, ts(b, 512)],
                            start=(ci == 0),
                            stop=(ci == 7),
                        )
                    nc.vector.tensor_scalar_add(qk_sb[:, ts(b, 512)], pa, bqk_sb)
                    # partition-swapped duplicate (SBUF->SBUF DMA, scalar queue
                    # so it never blocks the input stream)
                    nc.scalar.dma_start(
                        out=qk_swap[0:64, ts(b, 512)], in_=qk_sb[64:128, ts(b, 512)]
                    )
                    nc.scalar.dma_start(
                        out=qk_swap[64:128, ts(b, 512)], in_=qk_sb[0:64, ts(b, 512)]
                    )
                    # B: transposed conv -> qc^T | kc^T | ks^T for 4 n-subchunks.
                    # Even m-blocks store [qc|kc]; odd blocks store [kc|qc] so
                    # the full-slab DMA-transpose lands kc on the stacked half
                    # that out_sp won't overwrite.
                    swap = b % 2 == 1
                    qc_sl = slice(64, 128) if swap else slice(0, 64)
                    kc_sl = slice(0, 64) if swap else slice(64, 128)
                    for j in range(4):
                        nb = 4 * b + j
                        pb = psB.tile([128, 192], F32, tag="b", name="pb")
                        for ci in range(8):
                            nc.tensor.matmul(
                                pb,
                                chunks[ci][:, ts(nb, 128)],
                                wcsc_sb[:, ci, :],
                                start=(ci == 0),
                                stop=(ci == 7),
                            )
                        nc.vector.tensor_add(
                            qckcT[:, nb, qc_sl], pb[:, 0:64], bcsc_sb[:, 0:64]
                        )
                        nc.vector.tensor_add(
                            qckcT[:, nb, kc_sl], pb[:, 64:128], bcsc_sb[:, 64:128]
                        )
                        nc.vector.tensor_add(
                            ksT[:, nb, :], pb[:, 128:192], bcsc_sb[:, 128:192]
                        )
                        # channel-attn score accumulation (PSUM-resident)
                        nc.tensor.matmul(
                            sc_ps,
                            qckcT[:, nb, qc_sl],
                            qckcT[:, nb, kc_sl],
                            start=(nb == 0),
                            stop=(nb == 31),
                            skip_group_check=True,
                        )
                        # kc rows of `stacked` via PE transpose (qc half is
                        # junk there; overwritten later by the out_sp drains)
                        trp = psT.tile([128, 128], BF16, tag="t", name="trp")
                        nc.tensor.transpose(trp, qckcT[:, nb, :], ident)
                        nc.vector.tensor_copy(stacked[:, ts(nb, 128)], trp)

                # chan softmax + fold attn into final weights
                sc = chs.tile([64, 64], F32)
                nc.vector.tensor_copy(sc, sc_ps)
                mx = chs.tile([64, 1], F32)
                nc.vector.reduce_max(mx, sc, axis=AX)
                negmx = chs.tile([64, 1], F32)
                nc.vector.tensor_scalar_mul(negmx, mx, -1.0)
                ec = chs.tile([64, 64], F32)
                dc = chs.tile([64, 1], F32)
                nc.scalar.activation(ec, sc, EXP, bias=negmx, scale=1.0, accum_out=dc)
                rdc = chs.tile([64, 1], F32)
                nc.vector.reciprocal(rdc, dc)
                ac_bf = chs.tile([64, 64], BF16)
                nc.vector.tensor_scalar_mul(ac_bf, ec, rdc)
                psw = psW.tile([128, 512], F32, tag="w")
                nc.tensor.matmul(
                    psw[64:128, :], ac_bf, fcTa, start=True, stop=True,
                    skip_group_check=True,
                )
                nc.tensor.matmul(
                    psw[0:64, :], ac_bf, fcTb, start=True, stop=True,
                    skip_group_check=True,
                )
                nc.vector.tensor_copy(wf2a[64:128, :, :], psw[64:128, :])
                nc.vector.tensor_copy(wf2a[0:64, :, :], wfa_sb[0:64, :, :])
                nc.vector.tensor_copy(wf2b[0:64, :, :], psw[0:64, :])
                nc.vector.tensor_copy(wf2b[64:128, :, :], wfb_sb[64:128, :, :])

            # ---------------- spatial attention ----------------
            with (
                tc.tile_pool(name="spE", bufs=3) as spp,
                tc.tile_pool(name="spS", bufs=3) as sps,
                tc.tile_pool(name="psO", bufs=1, space="PSUM") as psO,
            ):
                out_ps = [
                    psO.tile([128, 512], F32, tag=f"o{j}", name=f"out_ps{j}")
                    for j in range(4)
                ]
                def emit_out_quarter(st, jj):
                    # 4 of the 16 deferred out-MMs for a finished pair
                    par, i_c, kst, E = st
                    for j in jj:
                        nc.tensor.matmul(
                            out_ps[j][0:64, :], kst, E[:, ts(2 * j, 512)],
                            start=(i_c == 0), stop=(i_c == 31),
                            skip_group_check=True,
                        )
                        nc.tensor.matmul(
                            out_ps[j][64:128, :],
                            kst,
                            E[:, ts(2 * j + 1, 512)],
                            start=(i_c == 0), stop=(i_c == 31),
                            skip_group_check=True,
                        )

                def emit_kst(par, i_c, dp, E):
                    d = sps.tile([128, 1], F32, tag=f"d{par}", name="d")
                    nc.vector.reduce_sum(d, dp, axis=AX)
                    rd = sps.tile([128, 1], F32, tag=f"rd{par}", name="rd")
                    nc.vector.reciprocal(rd, d)
                    kst = sps.tile([128, 64], BF16, tag=f"kst{par}", name="kst")
                    nc.vector.tensor_scalar_mul(kst, ksT[:, i_c, :], rd)
                    return (par, i_c, kst, E)

                # chunk pairs: even chunk on PE rows 0:64, odd on rows 64:128.
                # The contraction stage (out-MMs) for pair t-1 is interleaved
                # into pair t's S-matmul stream so the PE never blocks the
                # scalar exp pipeline at pair boundaries.
                prev = None  # (st_e, st_o) of previous pair
                with (
                    tc.tile_pool(name="psSe", bufs=1, space="PSUM") as psSe,
                    tc.tile_pool(name="psSo", bufs=1, space="PSUM") as psSo,
                ):
                  for t in range(16):
                    i_e, i_o = 2 * t, 2 * t + 1
                    E_e = spp.tile([128, HW], BF16, tag="Ee", name="E_e")
                    E_o = spp.tile([128, HW], BF16, tag="Eo", name="E_o")
                    dp_e = sps.tile([128, 4], F32, tag="dpe", name="dp_e")
                    dp_o = sps.tile([128, 4], F32, tag="dpo", name="dp_o")
                    for q in range(4):
                        s_e = psSe.tile([128, 1024], F32, tag="se", name="s_e")
                        s_o = psSo.tile([128, 1024], F32, tag="so", name="s_o")
                        for jm in range(2):
                            mb = 2 * q + jm
                            nc.tensor.matmul(
                                s_e[:, ts(jm, 512)],
                                qk_sb[0:64, ts(i_e, 128)],
                                qk_swap[0:64, ts(mb, 512)],
                                start=True,
                                stop=True,
                            )
                            nc.tensor.matmul(
                                s_o[:, ts(jm, 512)],
                                qk_swap[64:128, ts(i_o, 128)],
                                qk_sb[64:128, ts(mb, 512)],
                                start=True,
                                stop=True,
                            )
                        if prev is not None:
                            st = prev[0] if q < 2 else prev[1]
                            emit_out_quarter(st, (0, 1) if q % 2 == 0 else (2, 3))
                        nc.scalar.activation(
                            E_e[:, ts(q, 1024)], s_e, EXP,
                            accum_out=dp_e[:, q : q + 1],
                        )
                        if q in dve_qs:
                            # odd-parity tile: Horner-poly exp on the DVE
                            xb = sps.tile([128, 1024], BF16, tag="xb", name="xb")
                            nc.vector.tensor_copy(xb, s_o)
                            tA = sps.tile([128, 1024], BF16, tag="tA", name="tA")
                            tB = sps.tile([128, 1024], BF16, tag="tB", name="tB")
                            nc.vector.tensor_scalar_mul(tA, xb, c5)
                            for cc, src, dst in (
                                (c4, tA, tB), (c3, tB, tA),
                                (c2, tA, tB), (c1, tB, tA),
                            ):
                                nc.vector.scalar_tensor_tensor(
                                    dst, src, cc, xb,
                                    AluOpType.add, AluOpType.mult,
                                )
                            nc.vector.tensor_scalar(
                                E_o[:, ts(q, 1024)], tA, c0, 0.0,
                                AluOpType.add, AluOpType.add,
                                accum_out=dp_o[:, q : q + 1],
                            )
                        else:
                            if t == 15 and q == 3:
                                # last pair: e-parity contraction can start
                                # during the final o-ACT
                                st_e = emit_kst("e", i_e, dp_e, E_e)
                                emit_out_quarter(st_e, (0, 1, 2, 3))
                            nc.scalar.activation(
                                E_o[:, ts(q, 1024)], s_o, EXP,
                                accum_out=dp_o[:, q : q + 1],
                            )
                    if t < 15:
                        st_e = emit_kst("e", i_e, dp_e, E_e)
                    st_o = emit_kst("o", i_o, dp_o, E_o)
                    prev = (st_e, st_o)

                # tail: per j-block, flush the last pair's contraction, drain
                # out_ps to stacked, and run the final conv for its 2 m-blocks
                # (pipelines flush -> drain -> final instead of 3 barriers)
                with (
                    tc.tile_pool(name="fin", bufs=4) as fins,
                    tc.tile_pool(name="psF", bufs=4, space="PSUM") as psF,
                ):
                    out_r = out_d.rearrange("(k p) m -> k p m", p=128)
                    for j in range(4):
                        emit_out_quarter(prev[1], (j,))
                        nc.vector.tensor_copy(
                            stacked[0:64, ts(2 * j, 512)], out_ps[j][0:64, :]
                        )
                        nc.vector.tensor_copy(
                            stacked[64:128, ts(2 * j + 1, 512)],
                            out_ps[j][64:128, :],
                        )
                        for mb in (2 * j, 2 * j + 1):
                            wf = wf2a if mb % 2 == 0 else wf2b
                            for cok in range(4):
                                ps = psF.tile([128, 512], F32, tag="f")
                                nc.tensor.matmul(
                                    ps,
                                    wf[:, cok, :],
                                    stacked[:, ts(mb, 512)],
                                    start=True, stop=True,
                                )
                                ft = fins.tile([128, 512], BF16, tag="ft")
                                if (mb * 4 + cok) % 2 == 0:
                                    nc.vector.tensor_scalar_add(
                                        ft, ps, bf_sb[:, cok : cok + 1]
                                    )
                                    nc.sync.dma_start(
                                        out=out_r[cok, :, ts(mb, 512)], in_=ft
                                    )
                                else:
                                    nc.scalar.add(ft, ps, bf_sb[:, cok : cok + 1])
                                    nc.scalar.dma_start(
                                        out=out_r[cok, :, ts(mb, 512)], in_=ft
                                    )

    nc.compile()
    return nc


def make_weight_arrays(inputs):
    """Host-side composite weights (float64 accumulate, bf16/f32 out)."""
    f8 = lambda a: np.asarray(a, dtype=np.float64)
    wt, wb = f8(inputs["wt"]), f8(inputs["wb"])
    bt, bb = f8(inputs["bt"]), f8(inputs["bb"])
    s_w1, s_b1 = f8(inputs["s_w1"]), f8(inputs["s_b1"])
    s_w2, s_b2 = f8(inputs["s_w2"]), f8(inputs["s_b2"])
    s_wo, s_bo = f8(inputs["s_wo"]), f8(inputs["s_bo"])
    c_wq, c_bq = f8(inputs["c_wq"]), f8(inputs["c_bq"])
    c_wk, c_bk = f8(inputs["c_wk"]), f8(inputs["c_bk"])
    c_wo, c_bo = f8(inputs["c_wo"]), f8(inputs["c_bo"])
    f_w, f_b = f8(inputs["f_w"]), f8(inputs["f_b"])

    wt1, wt2 = wt[:CH], wt[CH:]
    wb1, wb2 = wb[:CH], wb[CH:]
    btb = bt + bb
    btb1, btb2 = btb[:CH], btb[CH:]

    A_q, B_q = s_w1 @ wt1, s_w1 @ wb1
    A_k, B_k = s_w2 @ wt1, s_w2 @ wb1
    C_q, D_q = c_wq @ wt2, c_wq @ wb2
    C_k, D_k = c_wk @ wt2, c_wk @ wb2

    wsp_full = np.concatenate(
        [
            np.concatenate([A_q.T, A_k.T], axis=1),
            np.concatenate([B_q.T, B_k.T], axis=1),
        ],
        axis=0,
    )  # [1024, 128]
    wsp = wsp_full.reshape(8, 128, 128).transpose(1, 0, 2)

    bias_q = s_w1 @ btb1 + s_b1
    bias_k = s_w2 @ btb1 + s_b2
    b_qk = np.concatenate([bias_q, bias_k])[:, None]

    wcsc_full = np.concatenate(
        [
            np.concatenate([C_q.T, C_k.T, A_k.T], axis=1),
            np.concatenate([D_q.T, D_k.T, B_k.T], axis=1),
        ],
        axis=0,
    )  # [1024, 192]
    wcsc = wcsc_full.reshape(8, 128, 192).transpose(1, 0, 2)

    bias_qc = c_wq @ btb2 + c_bq
    bias_kc = c_wk @ btb2 + c_bk
    bcsc_vec = np.concatenate([bias_qc, bias_kc, bias_k])  # [192]
    b_csc = np.broadcast_to(bcsc_vec, (128, 192)).copy()

    fs = f_w[:, :CH] @ s_wo  # [512, 64]
    fc = f_w[:, CH:] @ c_wo
    wfa = np.concatenate([fs, fc], axis=1).T.reshape(128, 4, 128)
    wfb = np.concatenate([fc, fs], axis=1).T.reshape(128, 4, 128)
    bias_f = f_w[:, :CH] @ s_bo + f_w[:, CH:] @ c_bo + f_b  # [512]
    b_f = bias_f.reshape(4, 128).T

    import ml_dtypes

    cast = lambda a: np.ascontiguousarray(a, dtype=np.float32)
    wcast = lambda a: np.ascontiguousarray(
        a.astype(np.float32), dtype=ml_dtypes.bfloat16
    )
    return {
        "wsp": wcast(wsp),
        "wcsc": wcast(wcsc),
        "wfa": wcast(wfa),
        "wfb": wcast(wfb),
        "b_qk": cast(b_qk),
        "b_csc": cast(b_csc),
        "b_f": cast(b_f),
    }


def kernel(**inputs):
    if "nc" not in _CACHE:
        _CACHE["nc"] = build_program()
    nc = _CACHE["nc"]

    import ml_dtypes

    weights = make_weight_arrays(inputs)
    top_all = np.ascontiguousarray(
        np.asarray(inputs["top_feat"], dtype=np.float32)
        .reshape(N_CORES, C, HW)
        .astype(ml_dtypes.bfloat16)
    )
    bot_all = np.ascontiguousarray(
        np.asarray(inputs["bottom_feat"], dtype=np.float32)
        .reshape(N_CORES, C, HW)
        .astype(ml_dtypes.bfloat16)
    )
    in_maps = [
        {"top": top_all[b], "bot": bot_all[b], **weights} for b in range(N_CORES)
    ]
    res = bass_utils.run_bass_kernel_spmd(nc, in_maps, core_ids=list(range(N_CORES)))
    _CACHE["last_res"] = res
    out = np.stack(
        [np.asarray(res.results[b]["out"], dtype=np.float32) for b in range(N_CORES)]
    )
    return out.reshape(N_CORES, C, 64, 64)


# revision 34
# speedup vs baseline: 1.1553x; 1.1553x over previous
"""CKAM (DANet-style dual attention) Bass kernel for 8 trn2 NeuronCores.

Data-parallel over batch: each core processes one [512, 64, 64] image.

Per-core dataflow (N = H*W = 4096, C = 512, CH = 256, R = 64):
  Head (column-streamed, overlaps the input DMA):
    per 512-col m-block b:
      A:  qk[:, b]   = Wsp^T @ x[:, b]          (spatial q|k, channel-major)
      B:  [qc|kc|ks]^T[b-rows] = x[:, b]^T @ Wcsc   (n-major)
      chan scores += qcT_nb^T @ kcT_nb          (PSUM-resident accumulation)
      kc rows of `stacked` via DMA-transpose of kcT
      qk_swap[:, b] partition-swapped dup of qk (SBUF-SBUF DMA)
  Chan tail: softmax(scores) -> ac; final weights folded on device:
      wf2 rows for the kc half become (fc @ ac)^T = ac^T @ fc^T  (1 matmul)
  Spatial attn: chunk pairs (even chunk on PE rows 0:64, odd on 64:128):
      S = q^T k -> exp (ACT, accum d; optionally DVE Horner poly for some
      tiles) -> out_sp += (ks^T / d) contracted with E (col-group pairs)
  Final: out = wf2 @ [out_sp | kc] + bias   (single K=128 bf16 conv), bf16 out.

All 1x1 convs are folded through the (never materialized) x = top+bottom:
composite weights are computed on the host in float64.
"""

import numpy as np

import concourse.bass as bass
import concourse.bacc as bacc
import concourse.mybir as mybir
import concourse.tile as tile
from concourse import bass_utils
from concourse.bass import ts
from concourse.alu_op_type import AluOpType
from concourse.masks import make_identity

N_CORES = 8
C, HW = 512, 4096
CH, R = 256, 64
F32 = mybir.dt.float32
BF16 = mybir.dt.bfloat16
EXP = mybir.ActivationFunctionType.Exp
AX = mybir.AxisListType.X

_CACHE: dict = {}

# which (o-parity) q-slots use the DVE polynomial exp instead of ScalarE ACT
# (measured: DVE STT runs at 1x here => poly is 5x slower than ACT; keep off)
DVE_EXP_QS: tuple = ()

# exp(x) ~ poly deg-5, density(sigma=0.4)-weighted fit with 1e-3 floor,
# valid on [-2.9, 2.9] (S scores are ~N(0, 0.35^2), |S| < 2.5 in practice)
def _exp_poly_coeffs():
    a = 2.9
    x = np.linspace(-a, a, 12001)
    dens = np.exp(-(x ** 2) / (2 * 0.40 ** 2))
    w = np.maximum(dens, 1e-3) / np.exp(x)
    c = np.polyfit(x, np.exp(x), 5, w=np.sqrt(w))
    return tuple(float(v) for v in c)  # highest degree first


def build_program(repeat=1, dve_qs=None):
    if dve_qs is None:
        dve_qs = DVE_EXP_QS
    c5, c4, c3, c2, c1, c0 = _exp_poly_coeffs()
    nc = bacc.Bacc("TRN2", target_bir_lowering=False, debug=False)

    top = nc.dram_tensor("top", (C, HW), BF16, kind="ExternalInput").ap()
    bot = nc.dram_tensor("bot", (C, HW), BF16, kind="ExternalInput").ap()
    wsp = nc.dram_tensor("wsp", (128, 8, 128), BF16, kind="ExternalInput").ap()
    wcsc = nc.dram_tensor("wcsc", (128, 8, 192), BF16, kind="ExternalInput").ap()
    wfa = nc.dram_tensor("wfa", (128, 4, 128), BF16, kind="ExternalInput").ap()
    wfb = nc.dram_tensor("wfb", (128, 4, 128), BF16, kind="ExternalInput").ap()
    b_qk = nc.dram_tensor("b_qk", (128, 1), F32, kind="ExternalInput").ap()
    b_csc = nc.dram_tensor("b_csc", (128, 192), F32, kind="ExternalInput").ap()
    b_f = nc.dram_tensor("b_f", (128, 4), F32, kind="ExternalInput").ap()
    out_d = nc.dram_tensor("out", (C, HW), BF16, kind="ExternalOutput").ap()

    with tile.TileContext(nc) as tc:
      for _rep in range(repeat):
        with (
            tc.tile_pool(name="consts", bufs=1) as consts,
            tc.tile_pool(name="persist", bufs=1) as persist,
        ):
            wsp_sb = consts.tile([128, 8, 128], BF16)
            nc.sync.dma_start(out=wsp_sb, in_=wsp)
            wcsc_sb = consts.tile([128, 8, 192], BF16)
            nc.sync.dma_start(out=wcsc_sb, in_=wcsc)
            bqk_sb = consts.tile([128, 1], F32)
            nc.sync.dma_start(out=bqk_sb, in_=b_qk)
            bcsc_sb = consts.tile([128, 192], F32)
            nc.sync.dma_start(out=bcsc_sb, in_=b_csc)
            wfa_sb = consts.tile([128, 4, 128], BF16)
            wfb_sb = consts.tile([128, 4, 128], BF16)
            bf_sb = consts.tile([128, 4], F32)
            fcTa = consts.tile([64, 4, 128], BF16)
            fcTb = consts.tile([64, 4, 128], BF16)
            ident = consts.tile([128, 128], BF16)
            make_identity(nc, ident)

            qk_sb = persist.tile([128, HW], BF16)  # q rows 0:64, k rows 64:128
            qk_swap = persist.tile([128, HW], BF16)  # [k | q] partition-swapped
            qckcT = persist.tile([128, 32, 128], BF16)  # qc^T | kc^T (n-major)
            ksT = persist.tile([128, 32, 64], BF16)  # spatial k^T
            stacked = persist.tile([128, HW], BF16)  # [out_sp | kc] final input
            wf2a = persist.tile([128, 4, 128], BF16)  # attn-folded final w
            wf2b = persist.tile([128, 4, 128], BF16)

            # ---------------- column-streamed head ----------------
            with (
                tc.tile_pool(name="inputs", bufs=1) as inputs,
                tc.tile_pool(name="psA", bufs=2, space="PSUM") as psA,
                tc.tile_pool(name="psB", bufs=2, space="PSUM") as psB,
                tc.tile_pool(name="psSc", bufs=1, space="PSUM") as psSc,
                tc.tile_pool(name="psW", bufs=1, space="PSUM") as psW,
                tc.tile_pool(name="psT", bufs=2, space="PSUM") as psT,
                tc.tile_pool(name="chs", bufs=1) as chs,
            ):
                top_c = top.rearrange("(a p) m -> p a m", p=128)
                bot_c = bot.rearrange("(a p) m -> p a m", p=128)
                top_sb = inputs.tile([128, 4, HW], BF16, name="top_sb")
                bot_sb = inputs.tile([128, 4, HW], BF16, name="bot_sb")
                chunks = [top_sb[:, a, :] for a in range(4)] + [
                    bot_sb[:, a, :] for a in range(4)
                ]
                sc_ps = psSc.tile([64, 64], F32, tag="sc")

                # input stream: one big DMA per (tensor, block), two HWDGE
                # queues in parallel, no compute interleaved in the queues
                for b in range(8):
                    nc.sync.dma_start(
                        out=top_sb[:, :, ts(b, 512)], in_=top_c[:, :, ts(b, 512)]
                    )
                    nc.scalar.dma_start(
                        out=bot_sb[:, :, ts(b, 512)], in_=bot_c[:, :, ts(b, 512)]
                    )
                # late-needed weights, behind the input streams
                nc.sync.dma_start(out=wfa_sb, in_=wfa)
                nc.sync.dma_start(out=wfb_sb, in_=wfb)
                nc.sync.dma_start(out=bf_sb, in_=b_f)
                nc.sync.dma_start(out=fcTa, in_=wfa_sb[64:128, :, :])
                nc.sync.dma_start(out=fcTb, in_=wfb_sb[0:64, :, :])

                for b in range(8):
                    # A: q|k conv for this column block
                    pa = psA.tile([128, 512], F32, tag="a", name="pa")
                    for ci in range(8):
                        nc.tensor.matmul(
                            pa,
                            wsp_sb[:, ci, :],
                            chunks[ci][:, ts(b, 512)],
                            start=(ci == 0),
                            stop=(ci == 7),
                        )
                    nc.vector.tensor_scalar_add(qk_sb[:, ts(b, 512)], pa, bqk_sb)
                    # partition-swapped duplicate (SBUF->SBUF DMA, scalar queue
                    # so it never blocks the input stream)
                    nc.scalar.dma_start(
                        out=qk_swap[0:64, ts(b, 512)], in_=qk_sb[64:128, ts(b, 512)]
                    )
                    nc.scalar.dma_start(
                        out=qk_swap[64:128, ts(b, 512)], in_=qk_sb[0:64, ts(b, 512)]
                    )
                    # B: transposed conv -> qc^T | kc^T | ks^T for 4 n-subchunks.
                    # Even m-blocks store [qc|kc]; odd blocks store [kc|qc] so
                    # the full-slab DMA-transpose lands kc on the stacked half
                    # that out_sp won't overwrite.
                    swap = b % 2 == 1
                    qc_sl = slice(64, 128) if swap else slice(0, 64)
                    kc_sl = slice(0, 64) if swap else slice(64, 128)
                    for j in range(4):
                        nb = 4 * b + j
                        pb = psB.tile([128, 192], F32, tag="b", name="pb")
                        for ci in range(8):
                            nc.tensor.matmul(
                                pb,
                                chunks[ci][:, ts(nb, 128)],
                                wcsc_sb[:, ci, :],
                                start=(ci == 0),
                                stop=(ci == 7),
                            )
                        nc.vector.tensor_add(
                            qckcT[:, nb, qc_sl], pb[:, 0:64], bcsc_sb[:, 0:64]
                        )
                        nc.vector.tensor_add(
                            qckcT[:, nb, kc_sl], pb[:, 64:128], bcsc_sb[:, 64:128]
                        )
                        nc.vector.tensor_add(
                            ksT[:, nb, :], pb[:, 128:192], bcsc_sb[:, 128:192]
                        )
                        # channel-attn score accumulation (PSUM-resident)
                        nc.tensor.matmul(
                            sc_ps,
                            qckcT[:, nb, qc_sl],
                            qckcT[:, nb, kc_sl],
                            start=(nb == 0),
                            stop=(nb == 31),
                            skip_group_check=True,
                        )
                        # kc rows of `stacked` via PE transpose (qc half is
                        # junk there; overwritten later by the out_sp drains)
                        trp = psT.tile([128, 128], BF16, tag="t", name="trp")
                        nc.tensor.transpose(trp, qckcT[:, nb, :], ident)
                        nc.vector.tensor_copy(stacked[:, ts(nb, 128)], trp)

                # chan softmax + fold attn into final weights
                sc = chs.tile([64, 64], F32)
                nc.vector.tensor_copy(sc, sc_ps)
                mx = chs.tile([64, 1], F32)
                nc.vector.reduce_max(mx, sc, axis=AX)
                negmx = chs.tile([64, 1], F32)
                nc.vector.tensor_scalar_mul(negmx, mx, -1.0)
                ec = chs.tile([64, 64], F32)
                dc = chs.tile([64, 1], F32)
                nc.scalar.activation(ec, sc, EXP, bias=negmx, scale=1.0, accum_out=dc)
                rdc = chs.tile([64, 1], F32)
                nc.vector.reciprocal(rdc, dc)
                ac_bf = chs.tile([64, 64], BF16)
                nc.vector.tensor_scalar_mul(ac_bf, ec, rdc)
                psw = psW.tile([128, 512], F32, tag="w")
                nc.tensor.matmul(
                    psw[64:128, :], ac_bf, fcTa, start=True, stop=True,
                    skip_group_check=True,
                )
                nc.tensor.matmul(
                    psw[0:64, :], ac_bf, fcTb, start=True, stop=True,
                    skip_group_check=True,
                )
                nc.vector.tensor_copy(wf2a[64:128, :, :], psw[64:128, :])
                nc.vector.tensor_copy(wf2a[0:64, :, :], wfa_sb[0:64, :, :])
                nc.vector.tensor_copy(wf2b[0:64, :, :], psw[0:64, :])
                nc.vector.tensor_copy(wf2b[64:128, :, :], wfb_sb[64:128, :, :])

            # ---------------- spatial attention ----------------
            with (
                tc.tile_pool(name="spE", bufs=2) as spp,
                tc.tile_pool(name="spS", bufs=2) as sps,
                tc.tile_pool(name="psO", bufs=1, space="PSUM") as psO,
            ):
                out_ps = [
                    psO.tile([128, 512], F32, tag=f"o{j}", name=f"out_ps{j}")
                    for j in range(4)
                ]
                def emit_out_quarter(st, jj):
                    # 4 of the 16 deferred out-MMs for a finished pair
                    par, i_c, kst, E = st
                    for j in jj:
                        nc.tensor.matmul(
                            out_ps[j][0:64, :], kst, E[:, ts(2 * j, 512)],
                            start=(i_c == 0), stop=(i_c == 31),
                            skip_group_check=True,
                        )
                        nc.tensor.matmul(
                            out_ps[j][64:128, :],
                            kst,
                            E[:, ts(2 * j + 1, 512)],
                            start=(i_c == 0), stop=(i_c == 31),
                            skip_group_check=True,
                        )

                def emit_kst(par, i_c, dp, E):
                    d = sps.tile([128, 1], F32, tag=f"d{par}", name="d")
                    nc.vector.reduce_sum(d, dp, axis=AX)
                    rd = sps.tile([128, 1], F32, tag=f"rd{par}", name="rd")
                    nc.vector.reciprocal(rd, d)
                    kst = sps.tile([128, 64], BF16, tag=f"kst{par}", name="kst")
                    nc.vector.tensor_scalar_mul(kst, ksT[:, i_c, :], rd)
                    return (par, i_c, kst, E)

                # chunk pairs: even chunk on PE rows 0:64, odd on rows 64:128.
                # The contraction stage (out-MMs) for pair t-1 is interleaved
                # into pair t's S-matmul stream so the PE never blocks the
                # scalar exp pipeline at pair boundaries.
                prev = None  # (st_e, st_o) of previous pair
                with (
                    tc.tile_pool(name="psSe", bufs=1, space="PSUM") as psSe,
                    tc.tile_pool(name="psSo", bufs=1, space="PSUM") as psSo,
                ):
                  for t in range(16):
                    i_e, i_o = 2 * t, 2 * t + 1
                    E_e = spp.tile([128, HW], BF16, tag="Ee", name="E_e")
                    E_o = spp.tile([128, HW], BF16, tag="Eo", name="E_o")
                    dp_e = sps.tile([128, 4], F32, tag="dpe", name="dp_e")
                    dp_o = sps.tile([128, 4], F32, tag="dpo", name="dp_o")
                    for q in range(4):
                        s_e = psSe.tile([128, 1024], F32, tag="se", name="s_e")
                        s_o = psSo.tile([128, 1024], F32, tag="so", name="s_o")
                        for jm in range(2):
                            mb = 2 * q + jm
                            nc.tensor.matmul(
                                s_e[:, ts(jm, 512)],
                                qk_sb[0:64, ts(i_e, 128)],
                                qk_swap[0:64, ts(mb, 512)],
                                start=True,
                                stop=True,
                            )
                            nc.tensor.matmul(
                                s_o[:, ts(jm, 512)],
                                qk_swap[64:128, ts(i_o, 128)],
                                qk_sb[64:128, ts(mb, 512)],
                                start=True,
                                stop=True,
                            )
                        if prev is not None:
                            st = prev[0] if q < 2 else prev[1]
                            emit_out_quarter(st, (0, 1) if q % 2 == 0 else (2, 3))
                        nc.scalar.activation(
                            E_e[:, ts(q, 1024)], s_e, EXP,
                            accum_out=dp_e[:, q : q + 1],
                        )
                        if q in dve_qs:
                            # odd-parity tile: Horner-poly exp on the DVE
                            xb = sps.tile([128, 1024], BF16, tag="xb", name="xb")
                            nc.vector.tensor_copy(xb, s_o)
                            tA = sps.tile([128, 1024], BF16, tag="tA", name="tA")
                            tB = sps.tile([128, 1024], BF16, tag="tB", name="tB")
                            nc.vector.tensor_scalar_mul(tA, xb, c5)
                            for cc, src, dst in (
                                (c4, tA, tB), (c3, tB, tA),
                                (c2, tA, tB), (c1, tB, tA),
                            ):
                                nc.vector.scalar_tensor_tensor(
                                    dst, src, cc, xb,
                                    AluOpType.add, AluOpType.mult,
                                )
                            nc.vector.tensor_scalar(
                                E_o[:, ts(q, 1024)], tA, c0, 0.0,
                                AluOpType.add, AluOpType.add,
                                accum_out=dp_o[:, q : q + 1],
                            )
                        else:
                            if t == 15 and q == 3:
                                # last pair: e-parity contraction can start
                                # during the final o-ACT
                                st_e = emit_kst("e", i_e, dp_e, E_e)
                                emit_out_quarter(st_e, (0, 1, 2, 3))
                            nc.scalar.activation(
                                E_o[:, ts(q, 1024)], s_o, EXP,
                                accum_out=dp_o[:, q : q + 1],
                            )
                    if t < 15:
                        st_e = emit_kst("e", i_e, dp_e, E_e)
                    st_o = emit_kst("o", i_o, dp_o, E_o)
                    prev = (st_e, st_o)

                # tail: per j-block, flush the last pair's contraction, drain
                # out_ps to stacked, and run the final conv for its 2 m-blocks
                # (pipelines flush -> drain -> final instead of 3 barriers)
                with (
                    tc.tile_pool(name="fin", bufs=4) as fins,
                    tc.tile_pool(name="psF", bufs=4, space="PSUM") as psF,
                ):
                    out_r = out_d.rearrange("(k p) m -> k p m", p=128)
                    for j in range(4):
                        emit_out_quarter(prev[1], (j,))
                        nc.vector.tensor_copy(
                            stacked[0:64, ts(2 * j, 512)], out_ps[j][0:64, :]
                        )
                        nc.vector.tensor_copy(
                            stacked[64:128, ts(2 * j + 1, 512)],
                            out_ps[j][64:128, :],
                        )
                        for mb in (2 * j, 2 * j + 1):
                            wf = wf2a if mb % 2 == 0 else wf2b
                            for cok in range(4):
                                ps = psF.tile([128, 512], F32, tag="f")
                                nc.tensor.matmul(
                                    ps,
                                    wf[:, cok, :],
                                    stacked[:, ts(mb, 512)],
                                    start=True, stop=True,
                                )
                                ft = fins.tile([128, 512], BF16, tag="ft")
                                if (mb * 4 + cok) % 2 == 0:
                                    nc.vector.tensor_scalar_add(
                                        ft, ps, bf_sb[:, cok : cok + 1]
                                    )
                                    nc.sync.dma_start(
                                        out=out_r[cok, :, ts(mb, 512)], in_=ft
                                    )
                                else:
                                    nc.scalar.add(ft, ps, bf_sb[:, cok : cok + 1])
                                    nc.scalar.dma_start(
                                        out=out_r[cok, :, ts(mb, 512)], in_=ft
                                    )

    nc.compile()
    return nc


def make_weight_arrays(inputs):
    """Host-side composite weights (float64 accumulate, bf16/f32 out)."""
    f8 = lambda a: np.asarray(a, dtype=np.float64)
    wt, wb = f8(inputs["wt"]), f8(inputs["wb"])
    bt, bb = f8(inputs["bt"]), f8(inputs["bb"])
    s_w1, s_b1 = f8(inputs["s_w1"]), f8(inputs["s_b1"])
    s_w2, s_b2 = f8(inputs["s_w2"]), f8(inputs["s_b2"])
    s_wo, s_bo = f8(inputs["s_wo"]), f8(inputs["s_bo"])
    c_wq, c_bq = f8(inputs["c_wq"]), f8(inputs["c_bq"])
    c_wk, c_bk = f8(inputs["c_wk"]), f8(inputs["c_bk"])
    c_wo, c_bo = f8(inputs["c_wo"]), f8(inputs["c_bo"])
    f_w, f_b = f8(inputs["f_w"]), f8(inputs["f_b"])

    wt1, wt2 = wt[:CH], wt[CH:]
    wb1, wb2 = wb[:CH], wb[CH:]
    btb = bt + bb
    btb1, btb2 = btb[:CH], btb[CH:]

    A_q, B_q = s_w1 @ wt1, s_w1 @ wb1
    A_k, B_k = s_w2 @ wt1, s_w2 @ wb1
    C_q, D_q = c_wq @ wt2, c_wq @ wb2
    C_k, D_k = c_wk @ wt2, c_wk @ wb2

    wsp_full = np.concatenate(
        [
            np.concatenate([A_q.T, A_k.T], axis=1),
            np.concatenate([B_q.T, B_k.T], axis=1),
        ],
        axis=0,
    )  # [1024, 128]
    wsp = wsp_full.reshape(8, 128, 128).transpose(1, 0, 2)

    bias_q = s_w1 @ btb1 + s_b1
    bias_k = s_w2 @ btb1 + s_b2
    b_qk = np.concatenate([bias_q, bias_k])[:, None]

    wcsc_full = np.concatenate(
        [
            np.concatenate([C_q.T, C_k.T, A_k.T], axis=1),
            np.concatenate([D_q.T, D_k.T, B_k.T], axis=1),
        ],
        axis=0,
    )  # [1024, 192]
    wcsc = wcsc_full.reshape(8, 128, 192).transpose(1, 0, 2)

    bias_qc = c_wq @ btb2 + c_bq
    bias_kc = c_wk @ btb2 + c_bk
    bcsc_vec = np.concatenate([bias_qc, bias_kc, bias_k])  # [192]
    b_csc = np.broadcast_to(bcsc_vec, (128, 192)).copy()

    fs = f_w[:, :CH] @ s_wo  # [512, 64]
    fc = f_w[:, CH:] @ c_wo
    wfa = np.concatenate([fs, fc], axis=1).T.reshape(128, 4, 128)
    wfb = np.concatenate([fc, fs], axis=1).T.reshape(128, 4, 128)
    bias_f = f_w[:, :CH] @ s_bo + f_w[:, CH:] @ c_bo + f_b  # [512]
    b_f = bias_f.reshape(4, 128).T

    import ml_dtypes

    cast = lambda a: np.ascontiguousarray(a, dtype=np.float32)
    wcast = lambda a: np.ascontiguousarray(
        a.astype(np.float32), dtype=ml_dtypes.bfloat16
    )
    return {
        "wsp": wcast(wsp),
        "wcsc": wcast(wcsc),
        "wfa": wcast(wfa),
        "wfb": wcast(wfb),
        "b_qk": cast(b_qk),
        "b_csc": cast(b_csc),
        "b_f": cast(b_f),
    }


def kernel(**inputs):
    if "nc" not in _CACHE:
        _CACHE["nc"] = build_program()
    nc = _CACHE["nc"]

    import ml_dtypes

    weights = make_weight_arrays(inputs)
    top_all = np.ascontiguousarray(
        np.asarray(inputs["top_feat"], dtype=np.float32)
        .reshape(N_CORES, C, HW)
        .astype(ml_dtypes.bfloat16)
    )
    bot_all = np.ascontiguousarray(
        np.asarray(inputs["bottom_feat"], dtype=np.float32)
        .reshape(N_CORES, C, HW)
        .astype(ml_dtypes.bfloat16)
    )
    in_maps = [
        {"top": top_all[b], "bot": bot_all[b], **weights} for b in range(N_CORES)
    ]
    res = bass_utils.run_bass_kernel_spmd(nc, in_maps, core_ids=list(range(N_CORES)))
    _CACHE["last_res"] = res
    out = np.stack(
        [np.asarray(res.results[b]["out"], dtype=np.float32) for b in range(N_CORES)]
    )
    return out.reshape(N_CORES, C, 64, 64)


# revision 36
# speedup vs baseline: 1.1627x; 1.0064x over previous
"""CKAM (DANet-style dual attention) Bass kernel for 8 trn2 NeuronCores.

Data-parallel over batch: each core processes one [512, 64, 64] image.

Per-core dataflow (N = H*W = 4096, C = 512, CH = 256, R = 64):
  Head (column-streamed, overlaps the input DMA):
    per 512-col m-block b:
      A:  qk[:, b]   = Wsp^T @ x[:, b]          (spatial q|k, channel-major)
      B:  [qc|kc|ks]^T[b-rows] = x[:, b]^T @ Wcsc   (n-major)
      chan scores += qcT_nb^T @ kcT_nb          (PSUM-resident accumulation)
      kc rows of `stacked` via DMA-transpose of kcT
      qk_swap[:, b] partition-swapped dup of qk (SBUF-SBUF DMA)
  Chan tail: softmax(scores) -> ac; final weights folded on device:
      wf2 rows for the kc half become (fc @ ac)^T = ac^T @ fc^T  (1 matmul)
  Spatial attn: chunk pairs (even chunk on PE rows 0:64, odd on 64:128):
      S = q^T k -> exp (ACT, accum d; optionally DVE Horner poly for some
      tiles) -> out_sp += (ks^T / d) contracted with E (col-group pairs)
  Final: out = wf2 @ [out_sp | kc] + bias   (single K=128 bf16 conv), bf16 out.

All 1x1 convs are folded through the (never materialized) x = top+bottom:
composite weights are computed on the host in float64.
"""

import numpy as np

import concourse.bass as bass
import concourse.bacc as bacc
import concourse.mybir as mybir
import concourse.tile as tile
from concourse import bass_utils
from concourse.bass import ts
from concourse.alu_op_type import AluOpType
from concourse.masks import make_identity

N_CORES = 8
C, HW = 512, 4096
CH, R = 256, 64
F32 = mybir.dt.float32
BF16 = mybir.dt.bfloat16
EXP = mybir.ActivationFunctionType.Exp
AX = mybir.AxisListType.X

_CACHE: dict = {}

# which (o-parity) q-slots use the DVE polynomial exp instead of ScalarE ACT
# (measured: DVE STT runs at 1x here => poly is 5x slower than ACT; keep off)
DVE_EXP_QS: tuple = ()

# exp(x) ~ poly deg-5, density(sigma=0.4)-weighted fit with 1e-3 floor,
# valid on [-2.9, 2.9] (S scores are ~N(0, 0.35^2), |S| < 2.5 in practice)
def _exp_poly_coeffs():
    a = 2.9
    x = np.linspace(-a, a, 12001)
    dens = np.exp(-(x ** 2) / (2 * 0.40 ** 2))
    w = np.maximum(dens, 1e-3) / np.exp(x)
    c = np.polyfit(x, np.exp(x), 5, w=np.sqrt(w))
    return tuple(float(v) for v in c)  # highest degree first


def build_program(repeat=1, dve_qs=None):
    if dve_qs is None:
        dve_qs = DVE_EXP_QS
    c5, c4, c3, c2, c1, c0 = _exp_poly_coeffs()
    nc = bacc.Bacc("TRN2", target_bir_lowering=False, debug=False)

    top = nc.dram_tensor("top", (C, HW), BF16, kind="ExternalInput").ap()
    bot = nc.dram_tensor("bot", (C, HW), BF16, kind="ExternalInput").ap()
    wsp = nc.dram_tensor("wsp", (128, 8, 128), BF16, kind="ExternalInput").ap()
    wcsc = nc.dram_tensor("wcsc", (128, 8, 192), BF16, kind="ExternalInput").ap()
    wfa = nc.dram_tensor("wfa", (128, 4, 128), BF16, kind="ExternalInput").ap()
    wfb = nc.dram_tensor("wfb", (128, 4, 128), BF16, kind="ExternalInput").ap()
    b_qk = nc.dram_tensor("b_qk", (128, 1), F32, kind="ExternalInput").ap()
    b_csc = nc.dram_tensor("b_csc", (128, 192), F32, kind="ExternalInput").ap()
    b_f = nc.dram_tensor("b_f", (128, 4), F32, kind="ExternalInput").ap()
    out_d = nc.dram_tensor("out", (C, HW), BF16, kind="ExternalOutput").ap()

    with tile.TileContext(nc) as tc:
      for _rep in range(repeat):
        with (
            tc.tile_pool(name="consts", bufs=1) as consts,
            tc.tile_pool(name="persist", bufs=1) as persist,
        ):
            wsp_sb = consts.tile([128, 8, 128], BF16)
            nc.sync.dma_start(out=wsp_sb, in_=wsp)
            wcsc_sb = consts.tile([128, 8, 192], BF16)
            nc.sync.dma_start(out=wcsc_sb, in_=wcsc)
            bqk_sb = consts.tile([128, 1], F32)
            nc.sync.dma_start(out=bqk_sb, in_=b_qk)
            bcsc_sb = consts.tile([128, 192], F32)
            nc.sync.dma_start(out=bcsc_sb, in_=b_csc)
            wfa_sb = consts.tile([128, 4, 128], BF16)
            wfb_sb = consts.tile([128, 4, 128], BF16)
            bf_sb = consts.tile([128, 4], F32)
            fcTa = consts.tile([64, 4, 128], BF16)
            fcTb = consts.tile([64, 4, 128], BF16)
            ident = consts.tile([128, 128], BF16)
            make_identity(nc, ident)

            qk_sb = persist.tile([128, HW], BF16)  # q rows 0:64, k rows 64:128
            qk_swap = persist.tile([128, HW], BF16)  # [k | q] partition-swapped
            qckcT = persist.tile([128, 32, 128], BF16)  # qc^T | kc^T (n-major)
            ksT = persist.tile([128, 32, 64], BF16)  # spatial k^T
            stacked = persist.tile([128, HW], BF16)  # [out_sp | kc] final input
            wf2a = persist.tile([128, 4, 128], BF16)  # attn-folded final w
            wf2b = persist.tile([128, 4, 128], BF16)

            # ---------------- column-streamed head ----------------
            with (
                tc.tile_pool(name="inputs", bufs=1) as inputs,
                tc.tile_pool(name="psA", bufs=2, space="PSUM") as psA,
                tc.tile_pool(name="psB", bufs=2, space="PSUM") as psB,
                tc.tile_pool(name="psSc", bufs=1, space="PSUM") as psSc,
                tc.tile_pool(name="psW", bufs=1, space="PSUM") as psW,
                tc.tile_pool(name="psT", bufs=2, space="PSUM") as psT,
                tc.tile_pool(name="chs", bufs=1) as chs,
            ):
                top_c = top.rearrange("(a p) m -> p a m", p=128)
                bot_c = bot.rearrange("(a p) m -> p a m", p=128)
                top_sb = inputs.tile([128, 4, HW], BF16, name="top_sb")
                bot_sb = inputs.tile([128, 4, HW], BF16, name="bot_sb")
                chunks = [top_sb[:, a, :] for a in range(4)] + [
                    bot_sb[:, a, :] for a in range(4)
                ]
                sc_ps = psSc.tile([64, 64], F32, tag="sc")

                # input stream: one big DMA per (tensor, block), two HWDGE
                # queues in parallel, no compute interleaved in the queues
                for b in range(8):
                    nc.sync.dma_start(
                        out=top_sb[:, :, ts(b, 512)], in_=top_c[:, :, ts(b, 512)]
                    )
                    nc.scalar.dma_start(
                        out=bot_sb[:, :, ts(b, 512)], in_=bot_c[:, :, ts(b, 512)]
                    )
                # late-needed weights, behind the input streams
                nc.sync.dma_start(out=wfa_sb, in_=wfa)
                nc.sync.dma_start(out=wfb_sb, in_=wfb)
                nc.sync.dma_start(out=bf_sb, in_=b_f)
                nc.sync.dma_start(out=fcTa, in_=wfa_sb[64:128, :, :])
                nc.sync.dma_start(out=fcTb, in_=wfb_sb[0:64, :, :])

                for b in range(8):
                    # A: q|k conv for this column block
                    pa = psA.tile([128, 512], F32, tag="a", name="pa")
                    for ci in range(8):
                        nc.tensor.matmul(
                            pa,
                            wsp_sb[:, ci, :],
                            chunks[ci][:, ts(b, 512)],
                            start=(ci == 0),
                            stop=(ci == 7),
                        )
                    nc.vector.tensor_scalar_add(qk_sb[:, ts(b, 512)], pa, bqk_sb)
                    # partition-swapped duplicate (SBUF->SBUF DMA, scalar queue
                    # so it never blocks the input stream)
                    nc.scalar.dma_start(
                        out=qk_swap[0:64, ts(b, 512)], in_=qk_sb[64:128, ts(b, 512)]
                    )
                    nc.scalar.dma_start(
                        out=qk_swap[64:128, ts(b, 512)], in_=qk_sb[0:64, ts(b, 512)]
                    )
                    # B: transposed conv -> qc^T | kc^T | ks^T for 4 n-subchunks.
                    # Even m-blocks store [qc|kc]; odd blocks store [kc|qc] so
                    # the full-slab DMA-transpose lands kc on the stacked half
                    # that out_sp won't overwrite.
                    swap = b % 2 == 1
                    qc_sl = slice(64, 128) if swap else slice(0, 64)
                    kc_sl = slice(0, 64) if swap else slice(64, 128)
                    for j in range(4):
                        nb = 4 * b + j
                        pb = psB.tile([128, 192], F32, tag="b", name="pb")
                        for ci in range(8):
                            nc.tensor.matmul(
                                pb,
                                chunks[ci][:, ts(nb, 128)],
                                wcsc_sb[:, ci, :],
                                start=(ci == 0),
                                stop=(ci == 7),
                            )
                        nc.vector.tensor_add(
                            qckcT[:, nb, qc_sl], pb[:, 0:64], bcsc_sb[:, 0:64]
                        )
                        nc.vector.tensor_add(
                            qckcT[:, nb, kc_sl], pb[:, 64:128], bcsc_sb[:, 64:128]
                        )
                        nc.vector.tensor_add(
                            ksT[:, nb, :], pb[:, 128:192], bcsc_sb[:, 128:192]
                        )
                        # channel-attn score accumulation (PSUM-resident)
                        nc.tensor.matmul(
                            sc_ps,
                            qckcT[:, nb, qc_sl],
                            qckcT[:, nb, kc_sl],
                            start=(nb == 0),
                            stop=(nb == 31),
                            skip_group_check=True,
                        )
                        # kc rows of `stacked` via PE transpose (qc half is
                        # junk there; overwritten later by the out_sp drains)
                        trp = psT.tile([128, 128], BF16, tag="t", name="trp")
                        nc.tensor.transpose(trp, qckcT[:, nb, :], ident)
                        nc.vector.tensor_copy(stacked[:, ts(nb, 128)], trp)

                # chan softmax + fold attn into final weights
                sc = chs.tile([64, 64], F32)
                nc.vector.tensor_copy(sc, sc_ps)
                mx = chs.tile([64, 1], F32)
                nc.vector.reduce_max(mx, sc, axis=AX)
                negmx = chs.tile([64, 1], F32)
                nc.vector.tensor_scalar_mul(negmx, mx, -1.0)
                ec = chs.tile([64, 64], F32)
                dc = chs.tile([64, 1], F32)
                nc.scalar.activation(ec, sc, EXP, bias=negmx, scale=1.0, accum_out=dc)
                rdc = chs.tile([64, 1], F32)
                nc.vector.reciprocal(rdc, dc)
                ac_bf = chs.tile([64, 64], BF16)
                nc.vector.tensor_scalar_mul(ac_bf, ec, rdc)
                psw = psW.tile([128, 512], F32, tag="w")
                nc.tensor.matmul(
                    psw[64:128, :], ac_bf, fcTa, start=True, stop=True,
                    skip_group_check=True,
                )
                nc.tensor.matmul(
                    psw[0:64, :], ac_bf, fcTb, start=True, stop=True,
                    skip_group_check=True,
                )
                nc.vector.tensor_copy(wf2a[64:128, :, :], psw[64:128, :])
                nc.vector.tensor_copy(wf2a[0:64, :, :], wfa_sb[0:64, :, :])
                nc.vector.tensor_copy(wf2b[0:64, :, :], psw[0:64, :])
                nc.vector.tensor_copy(wf2b[64:128, :, :], wfb_sb[64:128, :, :])

            # ---------------- spatial attention ----------------
            with (
                tc.tile_pool(name="spE", bufs=2) as spp,
                tc.tile_pool(name="spS", bufs=2) as sps,
                tc.tile_pool(name="psO", bufs=1, space="PSUM") as psO,
            ):
                out_ps = [
                    psO.tile([128, 512], F32, tag=f"o{j}", name=f"out_ps{j}")
                    for j in range(4)
                ]
                def emit_out_quarter(st, jj):
                    # 4 of the 16 deferred out-MMs for a finished pair
                    par, i_c, kst, E = st
                    for j in jj:
                        nc.tensor.matmul(
                            out_ps[j][0:64, :], kst, E[:, ts(2 * j, 512)],
                            start=(i_c == 0), stop=(i_c == 31),
                            skip_group_check=True,
                        )
                        nc.tensor.matmul(
                            out_ps[j][64:128, :],
                            kst,
                            E[:, ts(2 * j + 1, 512)],
                            start=(i_c == 0), stop=(i_c == 31),
                            skip_group_check=True,
                        )

                def emit_kst(par, i_c, dp, E):
                    d = sps.tile([128, 1], F32, tag=f"d{par}", name="d")
                    nc.vector.reduce_sum(d, dp, axis=AX)
                    rd = sps.tile([128, 1], F32, tag=f"rd{par}", name="rd")
                    nc.vector.reciprocal(rd, d)
                    kst = sps.tile([128, 64], BF16, tag=f"kst{par}", name="kst")
                    nc.vector.tensor_scalar_mul(kst, ksT[:, i_c, :], rd)
                    return (par, i_c, kst, E)

                # chunk pairs: even chunk on PE rows 0:64, odd on rows 64:128.
                # The contraction stage (out-MMs) for pair t-1 is interleaved
                # into pair t's S-matmul stream so the PE never blocks the
                # scalar exp pipeline at pair boundaries.
                prev = None  # (st_e, st_o) of previous pair
                with (
                    tc.tile_pool(name="psSe", bufs=1, space="PSUM") as psSe,
                    tc.tile_pool(name="psSo", bufs=1, space="PSUM") as psSo,
                ):
                  for t in range(16):
                    i_e, i_o = 2 * t, 2 * t + 1
                    E_e = spp.tile([128, HW], BF16, tag="Ee", name="E_e")
                    E_o = spp.tile([128, HW], BF16, tag="Eo", name="E_o")
                    dp_e = sps.tile([128, 4], F32, tag="dpe", name="dp_e")
                    dp_o = sps.tile([128, 4], F32, tag="dpo", name="dp_o")
                    for q in range(4):
                        s_e = psSe.tile([128, 1024], F32, tag="se", name="s_e")
                        s_o = psSo.tile([128, 1024], F32, tag="so", name="s_o")
                        for jm in range(2):
                            mb = 2 * q + jm
                            nc.tensor.matmul(
                                s_e[:, ts(jm, 512)],
                                qk_sb[0:64, ts(i_e, 128)],
                                qk_swap[0:64, ts(mb, 512)],
                                start=True,
                                stop=True,
                            )
                            nc.tensor.matmul(
                                s_o[:, ts(jm, 512)],
                                qk_swap[64:128, ts(i_o, 128)],
                                qk_sb[64:128, ts(mb, 512)],
                                start=True,
                                stop=True,
                            )
                        if prev is not None:
                            st = prev[0] if q < 2 else prev[1]
                            emit_out_quarter(st, (0, 1) if q % 2 == 0 else (2, 3))
                        nc.scalar.activation(
                            E_e[:, ts(q, 1024)], s_e, EXP,
                            accum_out=dp_e[:, q : q + 1],
                        )
                        if q in dve_qs:
                            # odd-parity tile: Horner-poly exp on the DVE
                            xb = sps.tile([128, 1024], BF16, tag="xb", name="xb")
                            nc.vector.tensor_copy(xb, s_o)
                            tA = sps.tile([128, 1024], BF16, tag="tA", name="tA")
                            tB = sps.tile([128, 1024], BF16, tag="tB", name="tB")
                            nc.vector.tensor_scalar_mul(tA, xb, c5)
                            for cc, src, dst in (
                                (c4, tA, tB), (c3, tB, tA),
                                (c2, tA, tB), (c1, tB, tA),
                            ):
                                nc.vector.scalar_tensor_tensor(
                                    dst, src, cc, xb,
                                    AluOpType.add, AluOpType.mult,
                                )
                            nc.vector.tensor_scalar(
                                E_o[:, ts(q, 1024)], tA, c0, 0.0,
                                AluOpType.add, AluOpType.add,
                                accum_out=dp_o[:, q : q + 1],
                            )
                        else:
                            nc.scalar.activation(
                                E_o[:, ts(q, 1024)], s_o, EXP,
                                accum_out=dp_o[:, q : q + 1],
                            )
                    st_e = emit_kst("e", i_e, dp_e, E_e)
                    st_o = emit_kst("o", i_o, dp_o, E_o)
                    prev = (st_e, st_o)

                # tail: per j-block, flush the last pair's contraction, drain
                # out_ps to stacked, and run the final conv for its 2 m-blocks
                # (pipelines flush -> drain -> final instead of 3 barriers)
                with (
                    tc.tile_pool(name="fin", bufs=4) as fins,
                    tc.tile_pool(name="psF", bufs=4, space="PSUM") as psF,
                ):
                    out_r = out_d.rearrange("(k p) m -> k p m", p=128)
                    for j in range(4):
                        emit_out_quarter(prev[0], (j,))
                        emit_out_quarter(prev[1], (j,))
                        nc.vector.tensor_copy(
                            stacked[0:64, ts(2 * j, 512)], out_ps[j][0:64, :]
                        )
                        nc.vector.tensor_copy(
                            stacked[64:128, ts(2 * j + 1, 512)],
                            out_ps[j][64:128, :],
                        )
                        for mb in (2 * j, 2 * j + 1):
                            wf = wf2a if mb % 2 == 0 else wf2b
                            for cok in range(4):
                                ps = psF.tile([128, 512], F32, tag="f")
                                nc.tensor.matmul(
                                    ps,
                                    wf[:, cok, :],
                                    stacked[:, ts(mb, 512)],
                                    start=True, stop=True,
                                )
                                ft = fins.tile([128, 512], BF16, tag="ft")
                                if (mb * 4 + cok) % 2 == 0:
                                    nc.vector.tensor_scalar_add(
                                        ft, ps, bf_sb[:, cok : cok + 1]
                                    )
                                    nc.sync.dma_start(
                                        out=out_r[cok, :, ts(mb, 512)], in_=ft
                                    )
                                else:
                                    nc.scalar.add(ft, ps, bf_sb[:, cok : cok + 1])
                                    nc.scalar.dma_start(
                                        out=out_r[cok, :, ts(mb, 512)], in_=ft
                                    )

    nc.compile()
    return nc


def make_weight_arrays(inputs):
    """Host-side composite weights (float64 accumulate, bf16/f32 out)."""
    f8 = lambda a: np.asarray(a, dtype=np.float64)
    wt, wb = f8(inputs["wt"]), f8(inputs["wb"])
    bt, bb = f8(inputs["bt"]), f8(inputs["bb"])
    s_w1, s_b1 = f8(inputs["s_w1"]), f8(inputs["s_b1"])
    s_w2, s_b2 = f8(inputs["s_w2"]), f8(inputs["s_b2"])
    s_wo, s_bo = f8(inputs["s_wo"]), f8(inputs["s_bo"])
    c_wq, c_bq = f8(inputs["c_wq"]), f8(inputs["c_bq"])
    c_wk, c_bk = f8(inputs["c_wk"]), f8(inputs["c_bk"])
    c_wo, c_bo = f8(inputs["c_wo"]), f8(inputs["c_bo"])
    f_w, f_b = f8(inputs["f_w"]), f8(inputs["f_b"])

    wt1, wt2 = wt[:CH], wt[CH:]
    wb1, wb2 = wb[:CH], wb[CH:]
    btb = bt + bb
    btb1, btb2 = btb[:CH], btb[CH:]

    A_q, B_q = s_w1 @ wt1, s_w1 @ wb1
    A_k, B_k = s_w2 @ wt1, s_w2 @ wb1
    C_q, D_q = c_wq @ wt2, c_wq @ wb2
    C_k, D_k = c_wk @ wt2, c_wk @ wb2

    wsp_full = np.concatenate(
        [
            np.concatenate([A_q.T, A_k.T], axis=1),
            np.concatenate([B_q.T, B_k.T], axis=1),
        ],
        axis=0,
    )  # [1024, 128]
    wsp = wsp_full.reshape(8, 128, 128).transpose(1, 0, 2)

    bias_q = s_w1 @ btb1 + s_b1
    bias_k = s_w2 @ btb1 + s_b2
    b_qk = np.concatenate([bias_q, bias_k])[:, None]

    wcsc_full = np.concatenate(
        [
            np.concatenate([C_q.T, C_k.T, A_k.T], axis=1),
            np.concatenate([D_q.T, D_k.T, B_k.T], axis=1),
        ],
        axis=0,
    )  # [1024, 192]
    wcsc = wcsc_full.reshape(8, 128, 192).transpose(1, 0, 2)

    bias_qc = c_wq @ btb2 + c_bq
    bias_kc = c_wk @ btb2 + c_bk
    bcsc_vec = np.concatenate([bias_qc, bias_kc, bias_k])  # [192]
    b_csc = np.broadcast_to(bcsc_vec, (128, 192)).copy()

    fs = f_w[:, :CH] @ s_wo  # [512, 64]
    fc = f_w[:, CH:] @ c_wo
    wfa = np.concatenate([fs, fc], axis=1).T.reshape(128, 4, 128)
    wfb = np.concatenate([fc, fs], axis=1).T.reshape(128, 4, 128)
    bias_f = f_w[:, :CH] @ s_bo + f_w[:, CH:] @ c_bo + f_b  # [512]
    b_f = bias_f.reshape(4, 128).T

    import ml_dtypes

    cast = lambda a: np.ascontiguousarray(a, dtype=np.float32)
    wcast = lambda a: np.ascontiguousarray(
        a.astype(np.float32), dtype=ml_dtypes.bfloat16
    )
    return {
        "wsp": wcast(wsp),
        "wcsc": wcast(wcsc),
        "wfa": wcast(wfa),
        "wfb": wcast(wfb),
        "b_qk": cast(b_qk),
        "b_csc": cast(b_csc),
        "b_f": cast(b_f),
    }


def kernel(**inputs):
    if "nc" not in _CACHE:
        _CACHE["nc"] = build_program()
    nc = _CACHE["nc"]

    import ml_dtypes

    weights = make_weight_arrays(inputs)
    top_all = np.ascontiguousarray(
        np.asarray(inputs["top_feat"], dtype=np.float32)
        .reshape(N_CORES, C, HW)
        .astype(ml_dtypes.bfloat16)
    )
    bot_all = np.ascontiguousarray(
        np.asarray(inputs["bottom_feat"], dtype=np.float32)
        .reshape(N_CORES, C, HW)
        .astype(ml_dtypes.bfloat16)
    )
    in_maps = [
        {"top": top_all[b], "bot": bot_all[b], **weights} for b in range(N_CORES)
    ]
    res = bass_utils.run_bass_kernel_spmd(nc, in_maps, core_ids=list(range(N_CORES)))
    _CACHE["last_res"] = res
    out = np.stack(
        [np.asarray(res.results[b]["out"], dtype=np.float32) for b in range(N_CORES)]
    )
    return out.reshape(N_CORES, C, 64, 64)
